# revision 8
# baseline (speedup 1.0000x reference)
"""Trainium2 Bass kernel for nn_MultiHeadAttention_89232240541956.

Computes, for B=8, S=4096, H=1024, ATTN=1024, EXT=1152:
    x_ext = [h | broadcast(g) | l]                       [B, S, 1152]
    q = relu(x_ext @ Wq + bq); k = relu(x_ext @ Wk + bk) [B, S, 1024]
    scores = sum(q * k, -1) / 32, masked to -1e9 where mask == 1

Sharding: data-parallel over batch — core b owns batch b.

Key transformations (baseline at 218us in kernel_baseline_218.py):
  - v (Wv, bv) is dead code in the reference — skipped.
  - g @ Wq[1024:1088] folded into the bias on host; bias folded into the
    matmul as a ones-row against a bias-row.
  - 6 matmul passes per projection per tile (vs 7): fp8 E4M3 DoubleRow
    chunks cover 706 of the 1089 contraction rows (3 passes: 256+256+194),
    bf16 covers the rest (3 passes: 128+128+127 = 318 h + 64 l + bias).
    n8=706 is the minimum fp8 coverage that reaches 6 passes, minimizing
    fp8 noise. Host-simulated (sim_err.py) max rel err 1.789e-2
    (device-validated sim); gate is 2e-2. Measured on device: 1.793e-2.
  - Partial-height chunks (97/127 partitions) matmul'd directly — no
    zero padding or wasted DMA bytes (matmul cost depends on N only).
  - Epilogue uses relu(q)*relu(k) == relu(relu(q)*k): ScalarE relu on q,
    DVE multiply vs raw k PSUM, ScalarE Relu-with-accum row reduction.
  - DMA discipline (the hard-won part): only sync/scalar trigger HWDGE
    (gpsimd software DGE crawls ~24 GB/s); per-ring throughput is only
    ~25-40 GB/s so bandwidth comes from many concurrent transfers; the
    head issues ~33 transfers in exact first-use order, alternating
    engines, with the first three (c0 W-q/W-k halves, block-0 fp8 x)
    split in half again for latency. fp8 W slabs are laid out nh-major
    so half-N transfers stay contiguous. x pools are triple-buffered
    and blocks 0-2 prefetch at the head (so their triggers don't queue
    behind epilogue work); later blocks trigger on the otherwise-idle
    sync queue.
  - Scales: x*16, W*64 (powers of two: lossless in bf16, in-range for
    TRN E4M3's +-240 max); 1/(32*2^20) folded into the epilogue accum.
"""

import numpy as np
import ml_dtypes

B, S, H, LOC = 8, 4096, 1024, 64
ATTN = 1024
SBLK = 512                # seq columns per DMA block
NBLK = S // SBLK          # 8
NT = SBLK // 128          # 4 seq tiles (128 tokens) per block
NCOL = S // 128           # 32 output columns

N8 = 706                  # h-dims in fp8 (2x256 full chunks + 194-row chunk)
NC2 = (N8 - 512) // 2     # 97 partitions in the short fp8 chunk
NBH = H - N8              # 318 bf16 h-dims
NJ2 = 62 + LOC + 1        # 127 partitions in the short bf16 chunk (h|l|bias)

BF16 = ml_dtypes.bfloat16
XS = 16.0
WS = 64.0

_CACHE = {}


def _build_nc():
    import concourse.bass as bass
    import concourse.mybir as mybir
    import concourse.tile as tile
    from concourse import bacc

    dt = mybir.dt
    nc = bacc.Bacc(None, target_bir_lowering=False)

    # fp8 x: [blk*128+p, (c*2+j)*512+s] for c<2; short chunk separate.
    xh8_d = nc.dram_tensor("xh8", [NBLK * 128, 2 * 2 * SBLK], dt.float8e4,
                           kind="ExternalInput")
    xh8c2_d = nc.dram_tensor("xh8c2", [NBLK * NC2, 2 * SBLK], dt.float8e4,
                             kind="ExternalInput")
    # bf16 x: [blk*128+p, j*512+s] for j<2; short chunk separate.
    xhb_d = nc.dram_tensor("xhb", [NBLK * 128, 2 * SBLK], dt.bfloat16,
                           kind="ExternalInput")
    xhbj2_d = nc.dram_tensor("xhbj2", [NBLK * NJ2, SBLK], dt.bfloat16,
                             kind="ExternalInput")
    # fp8 W, nh-major within a (c,proj) slab: [p, (c*2+proj, nh, j, a')]
    w8_d = nc.dram_tensor("w8", [128, 2 * 2 * 2 * ATTN], dt.float8e4,
                          kind="ExternalInput")
    w8c2_d = nc.dram_tensor("w8c2", [NC2, 2 * 2 * ATTN], dt.float8e4,
                            kind="ExternalInput")
    # bf16 W: [p, (j*2+proj)*1024 + a]  (a is naturally nh-major)
    wb_d = nc.dram_tensor("wb", [128, 2 * 2 * ATTN], dt.bfloat16,
                          kind="ExternalInput")
    wbj2_d = nc.dram_tensor("wbj2", [NJ2, 2 * ATTN], dt.bfloat16,
                            kind="ExternalInput")
    out = nc.dram_tensor("out", [128, NCOL], dt.float32, kind="ExternalOutput")

    scale = 1.0 / (32.0 * (XS * WS) ** 2)
    DR = mybir.MatmulPerfMode.DoubleRow
    Relu = mybir.ActivationFunctionType.Relu

    with tile.TileContext(nc) as tc:
        with (
            tc.tile_pool(name="wpool", bufs=1) as wpool,
            tc.tile_pool(name="xpool", bufs=3) as xpool,
            tc.tile_pool(name="epool", bufs=2) as epool,
            tc.tile_pool(name="opool", bufs=1) as opool,
            tc.tile_pool(name="psum", bufs=1, space="PSUM") as psum,
        ):
            # [p, c, proj, nh, j, a']
            w8_sb = wpool.tile([128, 2, 2, 2, 2, 512], dt.float8e4, tag="w8")
            w8c2_sb = wpool.tile([NC2, 2, 2, 2, 512], dt.float8e4, tag="w8c2")
            wb_sb = wpool.tile([128, 2, 2, ATTN], dt.bfloat16, tag="wb")
            wbj2_sb = wpool.tile([NJ2, 2, ATTN], dt.bfloat16, tag="wbj2")

            def make_x(blk):
                xh8 = xpool.tile([128, 2, 2, SBLK], dt.float8e4, tag="xh8",
                                 bufs=3, name=f"xh8_{blk}")
                xh8c2 = xpool.tile([NC2, 2, SBLK], dt.float8e4, tag="xh8c2",
                                   bufs=3, name=f"xh8c2_{blk}")
                xhb = xpool.tile([128, 2, SBLK], dt.bfloat16, tag="xhb",
                                 bufs=3, name=f"xhb_{blk}")
                xhbj2 = xpool.tile([NJ2, SBLK], dt.bfloat16, tag="xhbj2",
                                   bufs=3, name=f"xhbj2_{blk}")
                return xh8, xh8c2, xhb, xhbj2

            def dma_x(xt, blk, sync_only=False):
                xh8, xh8c2, xhb, xhbj2 = xt
                r0 = blk * 128
                e2 = nc.sync if sync_only else nc.scalar
                nc.sync.dma_start(xh8[:], xh8_d[r0:r0 + 128, :])
                e2.dma_start(xh8c2[:], xh8c2_d[blk * NC2:(blk + 1) * NC2, :])
                nc.sync.dma_start(xhb[:], xhb_d[r0:r0 + 128, :])
                e2.dma_start(xhbj2[:], xhbj2_d[blk * NJ2:(blk + 1) * NJ2, :])

            xt0 = make_x(0)
            xt1 = make_x(1)
            xt2 = make_x(2)
            xh8_0 = xt0[0]

            # --- head DMAs: exact first-use order, alternating the two
            # HWDGE engines; first three transfers split again for latency.
            S_, A_ = nc.sync, nc.scalar
            S_.dma_start(w8_sb[0:64, 0, 0, 0], w8_d[0:64, 0:1024])
            A_.dma_start(w8_sb[64:128, 0, 0, 0], w8_d[64:128, 0:1024])
            S_.dma_start(xh8_0[0:64, 0], xh8_d[0:64, 0:1024])
            A_.dma_start(xh8_0[64:128, 0], xh8_d[64:128, 0:1024])
            S_.dma_start(w8_sb[0:64, 0, 1, 0], w8_d[0:64, 2048:3072])
            A_.dma_start(w8_sb[64:128, 0, 1, 0], w8_d[64:128, 2048:3072])
            S_.dma_start(w8_sb[:, 0, 0, 1], w8_d[:, 1024:2048])
            A_.dma_start(w8_sb[:, 0, 1, 1], w8_d[:, 3072:4096])
            S_.dma_start(w8_sb[:, 1, 0, 0], w8_d[:, 4096:5120])
            A_.dma_start(w8_sb[:, 1, 1, 0], w8_d[:, 6144:7168])
            S_.dma_start(w8_sb[:, 1, 0, 1], w8_d[:, 5120:6144])
            A_.dma_start(w8_sb[:, 1, 1, 1], w8_d[:, 7168:8192])
            S_.dma_start(xh8_0[:, 1], xh8_d[0:128, 1024:2048])
            A_.dma_start(xt0[1][:], xh8c2_d[0:NC2, :])
            S_.dma_start(w8c2_sb[:, 0], w8c2_d[:, 0:2048])
            A_.dma_start(w8c2_sb[:, 1], w8c2_d[:, 2048:4096])
            S_.dma_start(xt0[2][:, 0], xhb_d[0:128, 0:512])
            A_.dma_start(xt0[2][:, 1], xhb_d[0:128, 512:1024])
            S_.dma_start(wb_sb[:, 0, 0], wb_d[:, 0:1024])
            A_.dma_start(wb_sb[:, 0, 1], wb_d[:, 1024:2048])
            S_.dma_start(xt0[3][:], xhbj2_d[0:NJ2, :])
            A_.dma_start(wb_sb[:, 1, 0], wb_d[:, 2048:3072])
            S_.dma_start(wb_sb[:, 1, 1], wb_d[:, 3072:4096])
            A_.dma_start(wbj2_sb[:, 0], wbj2_d[:, 0:1024])
            S_.dma_start(wbj2_sb[:, 1], wbj2_d[:, 1024:2048])
            # prefetch blocks 1 and 2
            dma_x(xt1, 1)
            dma_x(xt2, 2)

            score_sb = opool.tile([128, NCOL], dt.float32, tag="score")
            sc2 = opool.tile([128, 2], dt.float32, tag="sc2")

            def chunk_ops(xt, s0):
                """(lhs, rhs_q, rhs_k, perf_mode) per chunk, in order.
                rhs_* indexed as rhs[nh] -> AP."""
                xh8, xh8c2, xhb, xhbj2 = xt
                ops = []
                for c in range(2):
                    ops.append((xh8[:, c, :, s0:s0 + 128],
                                w8_sb[:, c, 0], w8_sb[:, c, 1], DR))
                ops.append((xh8c2[:, :, s0:s0 + 128],
                            w8c2_sb[:, 0], w8c2_sb[:, 1], DR))
                for j in range(2):
                    ops.append((xhb[:, j, s0:s0 + 128],
                                wb_sb[:, j, 0], wb_sb[:, j, 1], None))
                ops.append((xhbj2[:, s0:s0 + 128],
                            wbj2_sb[:, 0], wbj2_sb[:, 1], None))
                return ops

            def rsl(rhs, pm, nh):
                return rhs[:, nh] if pm else rhs[:, nh * 512:(nh + 1) * 512]

            x_tiles = {0: xt0, 1: xt1, 2: xt2}
            for blk in range(NBLK):
                if blk in x_tiles:
                    xt = x_tiles.pop(blk)
                else:
                    xt = make_x(blk)
                    dma_x(xt, blk, sync_only=True)

                for t in range(NT):
                    is_last = blk == NBLK - 1 and t == NT - 1
                    col = blk * NT + t
                    psq = psum.tile([128, ATTN], dt.float32, tag="psq",
                                    bufs=2, name=f"psq_{blk}_{t}")
                    psk = psum.tile([128, ATTN], dt.float32, tag="psk",
                                    bufs=2, name=f"psk_{blk}_{t}")
                    s0 = t * 128
                    ops = chunk_ops(xt, s0)

                    if not is_last:
                        for i, (lhs, rq, rk, pm) in enumerate(ops):
                            for nh in range(2):
                                n0 = nh * 512
                                nc.tensor.matmul(
                                    psq[:, n0:n0 + 512], lhs, rsl(rq, pm, nh),
                                    start=(i == 0), stop=(i == 5),
                                    perf_mode=pm)
                                nc.tensor.matmul(
                                    psk[:, n0:n0 + 512], lhs, rsl(rk, pm, nh),
                                    start=(i == 0), stop=(i == 5),
                                    perf_mode=pm)
                        qsb = epool.tile([128, ATTN], dt.bfloat16, tag="qsb")
                        nc.scalar.activation(qsb[:], psq[:], Relu)
                        prod = epool.tile([128, ATTN], dt.bfloat16, tag="prod")
                        nc.vector.tensor_mul(prod[:], qsb[:], psk[:])
                        cpy = epool.tile([128, ATTN], dt.bfloat16, tag="cpy")
                        nc.scalar.activation(
                            cpy[:], prod[:], Relu, scale=scale,
                            accum_out=score_sb[:, col:col + 1])
                        if col == NCOL - 5:
                            # early output slab once cols 0..27 are final
                            nc.sync.dma_start(out[:, 0:28], score_sb[:, 0:28])
                    else:
                        # q-pass fully first
                        for i, (lhs, rq, rk, pm) in enumerate(ops):
                            for nh in range(2):
                                nc.tensor.matmul(
                                    psq[:, nh * 512:nh * 512 + 512], lhs,
                                    rsl(rq, pm, nh),
                                    start=(i == 0), stop=(i == 5),
                                    perf_mode=pm)
                        # k-pass; q relu overlaps the k matmuls
                        qsb = epool.tile([128, ATTN], dt.bfloat16, tag="qsb")
                        nc.scalar.activation(qsb[:], psq[:], Relu)
                        for nh in range(2):
                            for i, (lhs, rq, rk, pm) in enumerate(ops):
                                nc.tensor.matmul(
                                    psk[:, nh * 512:nh * 512 + 512], lhs,
                                    rsl(rk, pm, nh),
                                    start=(i == 0), stop=(i == 5),
                                    perf_mode=pm)
                        for nh in range(2):
                            n0 = nh * 512
                            prh = epool.tile([128, 512], dt.bfloat16,
                                             tag="prh", name=f"prh_{nh}")
                            nc.vector.tensor_mul(prh[:], qsb[:, n0:n0 + 512],
                                                 psk[:, n0:n0 + 512])
                            cph = epool.tile([128, 512], dt.bfloat16,
                                             tag="cph", name=f"cph_{nh}")
                            nc.scalar.activation(
                                cph[:], prh[:], Relu, scale=scale,
                                accum_out=sc2[:, nh:nh + 1])
                        nc.vector.tensor_reduce(
                            score_sb[:, col:col + 1], sc2[:],
                            axis=mybir.AxisListType.X, op=mybir.AluOpType.add)
                        nc.sync.dma_start(out[:, 28:32], score_sb[:, 28:32])

    nc.compile()
    return nc


def _get_nc():
    if "nc" not in _CACHE:
        _CACHE["nc"] = _build_nc()
    return _CACHE["nc"]


def prep_in_maps(h, mask, g, l, Wq, bq, Wk, bk, Wv=None, bv=None):
    import concourse.mybir as mybir

    FP8 = mybir.dt.np(mybir.dt.float8e4)

    h = np.asarray(h, dtype=np.float32)
    g = np.asarray(g, dtype=np.float32)
    l_ = np.asarray(l, dtype=np.float32)
    Wq = np.asarray(Wq, dtype=np.float32)
    bq = np.asarray(bq, dtype=np.float32)
    Wk = np.asarray(Wk, dtype=np.float32)
    bk = np.asarray(bk, dtype=np.float32)

    # Fold the per-batch g contribution into the bias (fp32 on host).
    bq_eff = bq[None, :] + g @ Wq[H:H + LOC]            # [B, ATTN]
    bk_eff = bk[None, :] + g @ Wk[H:H + LOC]

    # --- shared weights ---
    w8 = np.empty((128, 2, 2, 2, 2, 512), dtype=FP8)    # [p,c,proj,nh,j,a']
    w8c2 = np.empty((NC2, 2, 2, 2, 512), dtype=FP8)     # [p,proj,nh,j,a']
    wb = np.empty((128, 2, 2, ATTN), dtype=BF16)        # [p,j,proj,a]
    wbj2_base = np.empty((NJ2, 2, ATTN), dtype=np.float32)
    for proj, W in ((0, Wq), (1, Wk)):
        W8 = (W[:N8] * WS).astype(FP8)
        # rows c*256+2p+j -> [c][p][j][nh][a'] -> [p][c][nh][j][a']
        w8[:, :, proj] = W8[:512].reshape(2, 128, 2, 2, 512).transpose(
            1, 0, 3, 2, 4)
        w8c2[:, proj] = W8[512:N8].reshape(NC2, 2, 2, 512).transpose(0, 2, 1, 3)
        Wbf = (W[N8:H] * WS).astype(BF16)
        wb[:, :, proj] = Wbf[:256].reshape(2, 128, ATTN).transpose(1, 0, 2)
        wbj2_base[0:62, proj] = W[N8 + 256:H] * WS
        wbj2_base[62:62 + LOC, proj] = W[H + LOC:] * WS
    base = {"w8": w8.reshape(128, -1), "w8c2": w8c2.reshape(NC2, -1),
            "wb": wb.reshape(128, -1)}

    in_maps = []
    for b in range(B):
        m = dict(base)
        hT = h[b].T                                     # [H, S]
        x8 = (hT[:N8] * XS).astype(FP8)                 # [706, S]
        # rows c*256+2p+j, cols blk*512+s -> [blk][p][c][j][s]
        m["xh8"] = np.ascontiguousarray(
            x8[:512].reshape(2, 128, 2, NBLK, SBLK).transpose(3, 1, 0, 2, 4)
        ).reshape(NBLK * 128, -1)
        m["xh8c2"] = np.ascontiguousarray(
            x8[512:N8].reshape(NC2, 2, NBLK, SBLK).transpose(2, 0, 1, 3)
        ).reshape(NBLK * NC2, -1)
        xb = (hT[N8:] * XS).astype(BF16)                # [318, S]
        m["xhb"] = np.ascontiguousarray(
            xb[:256].reshape(2, 128, NBLK, SBLK).transpose(2, 1, 0, 3)
        ).reshape(NBLK * 128, -1)
        xj2 = np.empty((NJ2, S), dtype=BF16)
        xj2[0:62] = xb[256:]
        xj2[62:62 + LOC] = l_[b].T * XS
        xj2[62 + LOC] = XS
        m["xhbj2"] = np.ascontiguousarray(
            xj2.reshape(NJ2, NBLK, SBLK).transpose(1, 0, 2)
        ).reshape(NBLK * NJ2, -1)
        wbj2 = wbj2_base.copy()
        # ones-row carries XS, so the bias row needs only WS.
        wbj2[62 + LOC, 0] = bq_eff[b] * WS
        wbj2[62 + LOC, 1] = bk_eff[b] * WS
        m["wbj2"] = wbj2.astype(BF16).reshape(NJ2, -1)
        in_maps.append(m)
    return in_maps


def kernel(h, mask, g, l, Wq, bq, Wk, bk, Wv=None, bv=None):
    from concourse.bass_utils import run_bass_kernel_spmd

    mask = np.asarray(mask)
    in_maps = prep_in_maps(h, mask, g, l, Wq, bq, Wk, bk)

    nc = _get_nc()
    res = run_bass_kernel_spmd(nc, in_maps, core_ids=list(range(B)), trace=False)

    scores = np.empty((B, S), dtype=np.float32)
    for b in range(B):
        scores[b] = res.results[b]["out"].T.reshape(S)
    return np.where(mask == 1, np.float32(-1e9), scores).astype(np.float32)


# revision 10
# speedup vs baseline: 1.0104x; 1.0104x over previous
"""Trainium2 Bass kernel for nn_MultiHeadAttention_89232240541956.

Computes, for B=8, S=4096, H=1024, ATTN=1024, EXT=1152:
    x_ext = [h | broadcast(g) | l]                       [B, S, 1152]
    q = relu(x_ext @ Wq + bq); k = relu(x_ext @ Wk + bk) [B, S, 1024]
    scores = sum(q * k, -1) / 32, masked to -1e9 where mask == 1

Sharding: data-parallel over batch — core b owns batch b.

Key transformations (baseline at 218us in kernel_baseline_218.py):
  - v (Wv, bv) is dead code in the reference — skipped.
  - g @ Wq[1024:1088] folded into the bias on host; bias folded into the
    matmul as a ones-row against a bias-row.
  - 6 matmul passes per projection per tile (vs 7): fp8 E4M3 DoubleRow
    chunks cover 706 of the 1089 contraction rows (3 passes: 256+256+194),
    bf16 covers the rest (3 passes: 128+128+127 = 318 h + 64 l + bias).
    n8=706 is the minimum fp8 coverage that reaches 6 passes, minimizing
    fp8 noise. Host-simulated (sim_err.py) max rel err 1.789e-2
    (device-validated sim); gate is 2e-2. Measured on device: 1.793e-2.
  - Partial-height chunks (97/127 partitions) matmul'd directly — no
    zero padding or wasted DMA bytes (matmul cost depends on N only).
  - Epilogue uses relu(q)*relu(k) == relu(relu(q)*k): ScalarE relu on q,
    DVE multiply vs raw k PSUM, ScalarE Relu-with-accum row reduction.
  - DMA discipline (the hard-won part): only sync/scalar trigger HWDGE
    (gpsimd software DGE crawls ~24 GB/s); per-ring throughput is only
    ~25-40 GB/s so bandwidth comes from many concurrent transfers; the
    head issues ~33 transfers in exact first-use order, alternating
    engines, with the first three (c0 W-q/W-k halves, block-0 fp8 x)
    split in half again for latency. fp8 W slabs are laid out nh-major
    so half-N transfers stay contiguous. x pools are triple-buffered
    and blocks 0-2 prefetch at the head (so their triggers don't queue
    behind epilogue work); later blocks trigger on the otherwise-idle
    sync queue.
  - Scales: x*16, W*64 (powers of two: lossless in bf16, in-range for
    TRN E4M3's +-240 max); 1/(32*2^20) folded into the epilogue accum.
"""

import numpy as np
import ml_dtypes

B, S, H, LOC = 8, 4096, 1024, 64
ATTN = 1024
SBLK = 512                # seq columns per DMA block
NBLK = S // SBLK          # 8
NT = SBLK // 128          # 4 seq tiles (128 tokens) per block
NCOL = S // 128           # 32 output columns

N8 = 706                  # h-dims in fp8 (2x256 full chunks + 194-row chunk)
NC2 = (N8 - 512) // 2     # 97 partitions in the short fp8 chunk
NBH = H - N8              # 318 bf16 h-dims
NJ2 = 62 + LOC + 1        # 127 partitions in the short bf16 chunk (h|l|bias)

BF16 = ml_dtypes.bfloat16
XS = 16.0
WS = 64.0

_CACHE = {}


def _build_nc():
    import concourse.bass as bass
    import concourse.mybir as mybir
    import concourse.tile as tile
    from concourse import bacc

    dt = mybir.dt
    nc = bacc.Bacc(None, target_bir_lowering=False)

    # fp8 x: [blk*128+p, (c*2+j)*512+s] for c<2; short chunk separate.
    xh8_d = nc.dram_tensor("xh8", [NBLK * 128, 2 * 2 * SBLK], dt.float8e4,
                           kind="ExternalInput")
    xh8c2_d = nc.dram_tensor("xh8c2", [NBLK * NC2, 2 * SBLK], dt.float8e4,
                             kind="ExternalInput")
    # bf16 x: [blk*128+p, j*512+s] for j<2; short chunk separate.
    xhb_d = nc.dram_tensor("xhb", [NBLK * 128, 2 * SBLK], dt.bfloat16,
                           kind="ExternalInput")
    xhbj2_d = nc.dram_tensor("xhbj2", [NBLK * NJ2, SBLK], dt.bfloat16,
                             kind="ExternalInput")
    # fp8 W, nh-major within a (c,proj) slab: [p, (c*2+proj, nh, j, a')]
    w8_d = nc.dram_tensor("w8", [128, 2 * 2 * 2 * ATTN], dt.float8e4,
                          kind="ExternalInput")
    w8c2_d = nc.dram_tensor("w8c2", [NC2, 2 * 2 * ATTN], dt.float8e4,
                            kind="ExternalInput")
    # bf16 W: [p, (j*2+proj)*1024 + a]  (a is naturally nh-major)
    wb_d = nc.dram_tensor("wb", [128, 2 * 2 * ATTN], dt.bfloat16,
                          kind="ExternalInput")
    wbj2_d = nc.dram_tensor("wbj2", [NJ2, 2 * ATTN], dt.bfloat16,
                            kind="ExternalInput")
    out = nc.dram_tensor("out", [128, NCOL], dt.float32, kind="ExternalOutput")

    scale = 1.0 / (32.0 * (XS * WS) ** 2)
    DR = mybir.MatmulPerfMode.DoubleRow
    Relu = mybir.ActivationFunctionType.Relu

    with tile.TileContext(nc) as tc:
        with (
            tc.tile_pool(name="wpool", bufs=1) as wpool,
            tc.tile_pool(name="xpool", bufs=3) as xpool,
            tc.tile_pool(name="epool", bufs=2) as epool,
            tc.tile_pool(name="opool", bufs=1) as opool,
            tc.tile_pool(name="psum", bufs=1, space="PSUM") as psum,
        ):
            # [p, c, proj, nh, j, a']
            w8_sb = wpool.tile([128, 2, 2, 2, 2, 512], dt.float8e4, tag="w8")
            w8c2_sb = wpool.tile([NC2, 2, 2, 2, 512], dt.float8e4, tag="w8c2")
            wb_sb = wpool.tile([128, 2, 2, ATTN], dt.bfloat16, tag="wb")
            wbj2_sb = wpool.tile([NJ2, 2, ATTN], dt.bfloat16, tag="wbj2")

            def make_x(blk):
                xh8 = xpool.tile([128, 2, 2, SBLK], dt.float8e4, tag="xh8",
                                 bufs=3, name=f"xh8_{blk}")
                xh8c2 = xpool.tile([NC2, 2, SBLK], dt.float8e4, tag="xh8c2",
                                   bufs=3, name=f"xh8c2_{blk}")
                xhb = xpool.tile([128, 2, SBLK], dt.bfloat16, tag="xhb",
                                 bufs=3, name=f"xhb_{blk}")
                xhbj2 = xpool.tile([NJ2, SBLK], dt.bfloat16, tag="xhbj2",
                                   bufs=3, name=f"xhbj2_{blk}")
                return xh8, xh8c2, xhb, xhbj2

            def dma_x(xt, blk, sync_only=False):
                xh8, xh8c2, xhb, xhbj2 = xt
                r0 = blk * 128
                e2 = nc.sync if sync_only else nc.scalar
                nc.sync.dma_start(xh8[:], xh8_d[r0:r0 + 128, :])
                e2.dma_start(xh8c2[:], xh8c2_d[blk * NC2:(blk + 1) * NC2, :])
                nc.sync.dma_start(xhb[:], xhb_d[r0:r0 + 128, :])
                e2.dma_start(xhbj2[:], xhbj2_d[blk * NJ2:(blk + 1) * NJ2, :])

            xt0 = make_x(0)
            xt1 = make_x(1)
            xt2 = make_x(2)
            xh8_0 = xt0[0]

            # --- head DMAs ---
            # While TensorE streams matmuls it throttles SBUF-bound DMA to
            # a ~25-35 GB/s trickle (SBUF port contention); with the
            # tensor idle, aggregate DMA is ~310 GB/s. So: load block-0 x
            # and ALL weights up front while the tensor waits (tile 0
            # consumes chunks in reverse arrival order, so its first
            # matmul is gated on the last weight slab), and let later x
            # blocks trickle in behind compute.
            S_, A_ = nc.sync, nc.scalar
            dma_x(xt0, 0)
            S_.dma_start(w8_sb[:, 0, 0], w8_d[:, 0:2048])
            A_.dma_start(w8_sb[:, 0, 1], w8_d[:, 2048:4096])
            S_.dma_start(w8_sb[:, 1, 0], w8_d[:, 4096:6144])
            A_.dma_start(w8_sb[:, 1, 1], w8_d[:, 6144:8192])
            S_.dma_start(w8c2_sb[:, 0], w8c2_d[:, 0:2048])
            A_.dma_start(w8c2_sb[:, 1], w8c2_d[:, 2048:4096])
            S_.dma_start(wb_sb[:, 0, 0], wb_d[:, 0:1024])
            A_.dma_start(wb_sb[:, 0, 1], wb_d[:, 1024:2048])
            S_.dma_start(wb_sb[:, 1, 0], wb_d[:, 2048:3072])
            A_.dma_start(wb_sb[:, 1, 1], wb_d[:, 3072:4096])
            S_.dma_start(wbj2_sb[:, 0], wbj2_d[:, 0:1024])
            A_.dma_start(wbj2_sb[:, 1], wbj2_d[:, 1024:2048])
            # prefetch blocks 1 and 2 behind the weights
            dma_x(xt1, 1)
            dma_x(xt2, 2)

            score_sb = opool.tile([128, NCOL], dt.float32, tag="score")
            sc2 = opool.tile([128, 2], dt.float32, tag="sc2")

            def chunk_ops(xt, s0):
                """(lhs, rhs_q, rhs_k, perf_mode) per chunk, in order.
                rhs_* indexed as rhs[nh] -> AP."""
                xh8, xh8c2, xhb, xhbj2 = xt
                ops = []
                for c in range(2):
                    ops.append((xh8[:, c, :, s0:s0 + 128],
                                w8_sb[:, c, 0], w8_sb[:, c, 1], DR))
                ops.append((xh8c2[:, :, s0:s0 + 128],
                            w8c2_sb[:, 0], w8c2_sb[:, 1], DR))
                for j in range(2):
                    ops.append((xhb[:, j, s0:s0 + 128],
                                wb_sb[:, j, 0], wb_sb[:, j, 1], None))
                ops.append((xhbj2[:, s0:s0 + 128],
                            wbj2_sb[:, 0], wbj2_sb[:, 1], None))
                return ops

            def rsl(rhs, pm, nh):
                return rhs[:, nh] if pm else rhs[:, nh * 512:(nh + 1) * 512]

            x_tiles = {0: xt0, 1: xt1, 2: xt2}
            for blk in range(NBLK):
                if blk in x_tiles:
                    xt = x_tiles.pop(blk)
                else:
                    xt = make_x(blk)
                    dma_x(xt, blk, sync_only=True)

                for t in range(NT):
                    is_last = blk == NBLK - 1 and t == NT - 1
                    col = blk * NT + t
                    psq = psum.tile([128, ATTN], dt.float32, tag="psq",
                                    bufs=2, name=f"psq_{blk}_{t}")
                    psk = psum.tile([128, ATTN], dt.float32, tag="psk",
                                    bufs=2, name=f"psk_{blk}_{t}")
                    s0 = t * 128
                    ops = chunk_ops(xt, s0)

                    if not is_last:
                        # tile (0,0) runs chunks in reverse arrival order so
                        # its first matmul waits for the last weight slab —
                        # the full-W gate that keeps DMA in burst mode.
                        order = ops[::-1] if col == 0 else ops
                        for i, (lhs, rq, rk, pm) in enumerate(order):
                            for nh in range(2):
                                n0 = nh * 512
                                nc.tensor.matmul(
                                    psq[:, n0:n0 + 512], lhs, rsl(rq, pm, nh),
                                    start=(i == 0), stop=(i == 5),
                                    perf_mode=pm)
                                nc.tensor.matmul(
                                    psk[:, n0:n0 + 512], lhs, rsl(rk, pm, nh),
                                    start=(i == 0), stop=(i == 5),
                                    perf_mode=pm)
                        qsb = epool.tile([128, ATTN], dt.bfloat16, tag="qsb")
                        nc.scalar.activation(qsb[:], psq[:], Relu)
                        prod = epool.tile([128, ATTN], dt.bfloat16, tag="prod")
                        nc.vector.tensor_mul(prod[:], qsb[:], psk[:])
                        cpy = epool.tile([128, ATTN], dt.bfloat16, tag="cpy")
                        nc.scalar.activation(
                            cpy[:], prod[:], Relu, scale=scale,
                            accum_out=score_sb[:, col:col + 1])
                        if col == NCOL - 5:
                            # early output slab once cols 0..27 are final
                            nc.sync.dma_start(out[:, 0:28], score_sb[:, 0:28])
                    else:
                        # q-pass fully first
                        for i, (lhs, rq, rk, pm) in enumerate(ops):
                            for nh in range(2):
                                nc.tensor.matmul(
                                    psq[:, nh * 512:nh * 512 + 512], lhs,
                                    rsl(rq, pm, nh),
                                    start=(i == 0), stop=(i == 5),
                                    perf_mode=pm)
                        # k-pass; q relu overlaps the k matmuls
                        qsb = epool.tile([128, ATTN], dt.bfloat16, tag="qsb")
                        nc.scalar.activation(qsb[:], psq[:], Relu)
                        for nh in range(2):
                            for i, (lhs, rq, rk, pm) in enumerate(ops):
                                nc.tensor.matmul(
                                    psk[:, nh * 512:nh * 512 + 512], lhs,
                                    rsl(rk, pm, nh),
                                    start=(i == 0), stop=(i == 5),
                                    perf_mode=pm)
                        for nh in range(2):
                            n0 = nh * 512
                            prh = epool.tile([128, 512], dt.bfloat16,
                                             tag="prh", name=f"prh_{nh}")
                            nc.vector.tensor_mul(prh[:], qsb[:, n0:n0 + 512],
                                                 psk[:, n0:n0 + 512])
                            cph = epool.tile([128, 512], dt.bfloat16,
                                             tag="cph", name=f"cph_{nh}")
                            nc.scalar.activation(
                                cph[:], prh[:], Relu, scale=scale,
                                accum_out=sc2[:, nh:nh + 1])
                        nc.vector.tensor_reduce(
                            score_sb[:, col:col + 1], sc2[:],
                            axis=mybir.AxisListType.X, op=mybir.AluOpType.add)
                        nc.sync.dma_start(out[:, 28:32], score_sb[:, 28:32])

    nc.compile()
    return nc


def _get_nc():
    if "nc" not in _CACHE:
        _CACHE["nc"] = _build_nc()
    return _CACHE["nc"]


def prep_in_maps(h, mask, g, l, Wq, bq, Wk, bk, Wv=None, bv=None):
    import concourse.mybir as mybir

    FP8 = mybir.dt.np(mybir.dt.float8e4)

    h = np.asarray(h, dtype=np.float32)
    g = np.asarray(g, dtype=np.float32)
    l_ = np.asarray(l, dtype=np.float32)
    Wq = np.asarray(Wq, dtype=np.float32)
    bq = np.asarray(bq, dtype=np.float32)
    Wk = np.asarray(Wk, dtype=np.float32)
    bk = np.asarray(bk, dtype=np.float32)

    # Fold the per-batch g contribution into the bias (fp32 on host).
    bq_eff = bq[None, :] + g @ Wq[H:H + LOC]            # [B, ATTN]
    bk_eff = bk[None, :] + g @ Wk[H:H + LOC]

    # --- shared weights ---
    w8 = np.empty((128, 2, 2, 2, 2, 512), dtype=FP8)    # [p,c,proj,nh,j,a']
    w8c2 = np.empty((NC2, 2, 2, 2, 512), dtype=FP8)     # [p,proj,nh,j,a']
    wb = np.empty((128, 2, 2, ATTN), dtype=BF16)        # [p,j,proj,a]
    wbj2_base = np.empty((NJ2, 2, ATTN), dtype=np.float32)
    for proj, W in ((0, Wq), (1, Wk)):
        W8 = (W[:N8] * WS).astype(FP8)
        # rows c*256+2p+j -> [c][p][j][nh][a'] -> [p][c][nh][j][a']
        w8[:, :, proj] = W8[:512].reshape(2, 128, 2, 2, 512).transpose(
            1, 0, 3, 2, 4)
        w8c2[:, proj] = W8[512:N8].reshape(NC2, 2, 2, 512).transpose(0, 2, 1, 3)
        Wbf = (W[N8:H] * WS).astype(BF16)
        wb[:, :, proj] = Wbf[:256].reshape(2, 128, ATTN).transpose(1, 0, 2)
        wbj2_base[0:62, proj] = W[N8 + 256:H] * WS
        wbj2_base[62:62 + LOC, proj] = W[H + LOC:] * WS
    base = {"w8": w8.reshape(128, -1), "w8c2": w8c2.reshape(NC2, -1),
            "wb": wb.reshape(128, -1)}

    in_maps = []
    for b in range(B):
        m = dict(base)
        hT = h[b].T                                     # [H, S]
        x8 = (hT[:N8] * XS).astype(FP8)                 # [706, S]
        # rows c*256+2p+j, cols blk*512+s -> [blk][p][c][j][s]
        m["xh8"] = np.ascontiguousarray(
            x8[:512].reshape(2, 128, 2, NBLK, SBLK).transpose(3, 1, 0, 2, 4)
        ).reshape(NBLK * 128, -1)
        m["xh8c2"] = np.ascontiguousarray(
            x8[512:N8].reshape(NC2, 2, NBLK, SBLK).transpose(2, 0, 1, 3)
        ).reshape(NBLK * NC2, -1)
        xb = (hT[N8:] * XS).astype(BF16)                # [318, S]
        m["xhb"] = np.ascontiguousarray(
            xb[:256].reshape(2, 128, NBLK, SBLK).transpose(2, 1, 0, 3)
        ).reshape(NBLK * 128, -1)
        xj2 = np.empty((NJ2, S), dtype=BF16)
        xj2[0:62] = xb[256:]
        xj2[62:62 + LOC] = l_[b].T * XS
        xj2[62 + LOC] = XS
        m["xhbj2"] = np.ascontiguousarray(
            xj2.reshape(NJ2, NBLK, SBLK).transpose(1, 0, 2)
        ).reshape(NBLK * NJ2, -1)
        wbj2 = wbj2_base.copy()
        # ones-row carries XS, so the bias row needs only WS.
        wbj2[62 + LOC, 0] = bq_eff[b] * WS
        wbj2[62 + LOC, 1] = bk_eff[b] * WS
        m["wbj2"] = wbj2.astype(BF16).reshape(NJ2, -1)
        in_maps.append(m)
    return in_maps


def kernel(h, mask, g, l, Wq, bq, Wk, bk, Wv=None, bv=None):
    from concourse.bass_utils import run_bass_kernel_spmd

    mask = np.asarray(mask)
    in_maps = prep_in_maps(h, mask, g, l, Wq, bq, Wk, bk)

    nc = _get_nc()
    res = run_bass_kernel_spmd(nc, in_maps, core_ids=list(range(B)), trace=False)

    scores = np.empty((B, S), dtype=np.float32)
    for b in range(B):
        scores[b] = res.results[b]["out"].T.reshape(S)
    return np.where(mask == 1, np.float32(-1e9), scores).astype(np.float32)
